# revision 8
# baseline (speedup 1.0000x reference)
"""Trainium2 Bass kernel for nn_CorePartLayer.

Computes: proj = (L * z) @ U + mu -> (B, DIM); reshaped to (B, C, 32, 32, 32)
and placed at offset 16 on each spatial axis inside a zero (B, C, 64, 64, 64)
output.

Sharding: one channel per NeuronCore (DIM = C * 32^3 and C == n_cores == 8).
Core c computes the full-batch projection for its channel's 32768 columns.

The problem is memory-bound (per-core: read U slice + write projection), so the
kernel minimizes HBM bytes:
  - U is quantized on the host: basis rows 0..31 (large |L|) in fp16, rows
    32..63 (small |L|) in fp8 e4m3 scaled by 256 (the 1/256 is folded into the
    lo half of lhsT).  3MB read per core instead of 8MB f32; quantization puts
    ~1e-2 relative error on the result, well under the 2e-2 gate.
  - Data is pre-scrambled into per-chunk (64, cols) tensors: partition
    p = 32*h + r holds quantized row r for column half h, so chunks are fully
    contiguous DRAM regions.
  - lhsT = (L*z).T is precomputed on the host (fp16); mu (zero in practice) is
    applied host-side.
  - The projection is written compact in fp16 (2MB instead of 8MB of padded
    f32); the host unscrambles and places it into the zero (64,64,64) volume
    during the gather step.
  - Matmuls run with PE row+column tiling (tile_position=(32h, 32j)); the fp8
    matmul accumulates onto the fp16 one in PSUM (standard K-tile chain).

Pipeline shaping: chunk sizes descend so the tail chunk's compute+store is
short; every chunk gets its own SBUF buffer (all U loads queue upfront and the
HWDGE rings drain them back to back); hi loads ride the ACT ring, lo loads the
SP ring; PSUM drain copies rotate across DVE/ACT; chunk stores alternate
between the sync HWDGE ring and the gpsimd SWDGE ring.
"""

from contextlib import ExitStack

import numpy as np

import concourse.bass as bass
import concourse.tile as tile
from concourse import bacc, mybir
from concourse.bass_utils import run_bass_kernel_spmd

B = 32          # batch
NB = 64         # n_basis (contraction)
C = 8           # channels == n_cores
CORE = 32       # core cube edge
RES = 64        # output cube edge
POS = 16        # placement offset
CPD = CORE * CORE * CORE  # columns per channel = 32768
HALF = CPD // 2           # 16384 columns per partition-half
CHUNKS = [4096, 4096, 4096, 2048, 2048]  # columns per half, per chunk
assert sum(CHUNKS) == HALF
MIXED_FP8 = True  # rows 32..63 of U in fp8 e4m3 (x256); False = all fp16
K0 = 32           # fp16 rows when MIXED_FP8
LO_SCALE = 256.0
F16 = mybir.dt.float16
F8 = mybir.dt.float8e4
F32 = mybir.dt.float32

_NC_CACHE = {}


def _emit(ctx, tc):
    nc = tc.nc
    if MIXED_FP8:
        lhs_hi_d = nc.dram_tensor("lhsT_hi", [2 * K0, B], F16, kind="ExternalInput").ap()
        lhs_lo_d = nc.dram_tensor("lhsT_lo", [2 * K0, B], F16, kind="ExternalInput").ap()
        Uhi_d = [
            nc.dram_tensor(f"Uhi{g}", [2 * K0, w], F16, kind="ExternalInput").ap()
            for g, w in enumerate(CHUNKS)
        ]
        Ulo_d = [
            nc.dram_tensor(f"Ulo{g}", [2 * K0, w], F8, kind="ExternalInput").ap()
            for g, w in enumerate(CHUNKS)
        ]
    else:
        lhs_d = nc.dram_tensor("lhsT", [2 * NB, B], F16, kind="ExternalInput").ap()
        U_d = [
            nc.dram_tensor(f"U{g}", [128, w], F16, kind="ExternalInput").ap()
            for g, w in enumerate(CHUNKS)
        ]
    O_d = [
        nc.dram_tensor(f"O{g}", [128, w // 2], F16, kind="ExternalOutput").ap()
        for g, w in enumerate(CHUNKS)
    ]

    const = ctx.enter_context(tc.tile_pool(name="const", bufs=1))
    upool = ctx.enter_context(tc.tile_pool(name="u", bufs=1))
    spool = ctx.enter_context(tc.tile_pool(name="stage", bufs=1))
    pmm = ctx.enter_context(tc.tile_pool(name="pmm", bufs=8, space="PSUM"))

    if MIXED_FP8:
        lhs_hi = const.tile([2 * K0, B], F16, tag="lhsT_hi")
        lhs_lo = const.tile([2 * K0, B], F16, tag="lhsT_lo")
        nc.sync.dma_start(lhs_hi[:, :], lhs_hi_d)
        nc.sync.dma_start(lhs_lo[:, :], lhs_lo_d)
    else:
        lhsT = const.tile([2 * NB, B], F16, tag="lhsT")
        nc.sync.dma_start(lhsT[:, :], lhs_d)

    # Queue every U load upfront; each chunk has its own buffer so the DGE
    # rings stream the whole read set back to back with no dependency stalls.
    u_tiles = []
    for g, w in enumerate(CHUNKS):
        if MIXED_FP8:
            uhi = upool.tile([2 * K0, w], F16, tag=f"uhi{g}")
            ulo = upool.tile([2 * K0, w], F8, tag=f"ulo{g}")
            nc.scalar.dma_start(uhi[:, :], Uhi_d[g])
            nc.sync.dma_start(ulo[:, :], Ulo_d[g])
            u_tiles.append((uhi, ulo))
        else:
            u2 = upool.tile([128, w], F16, tag=f"u{g}")
            nc.scalar.dma_start(u2[:, :], U_d[g])
            u_tiles.append(u2)

    copy_engines = [nc.vector, nc.scalar]
    tcount = 0
    for g, w in enumerate(CHUNKS):
        NQ = w // 2048
        S = spool.tile([128, w // 2], F16, tag=f"s{g}")
        for q in range(NQ):
            for h in range(2):
                P = pmm.tile([128, 512], F32, tag="mm")
                for j in range(4):
                    f0 = 2048 * q + 512 * j
                    if MIXED_FP8:
                        uhi, ulo = u_tiles[g]
                        nc.tensor.matmul(
                            P[32 * j : 32 * j + 32, :],
                            lhs_hi[K0 * h : K0 * h + K0, :],
                            uhi[K0 * h : K0 * h + K0, f0 : f0 + 512],
                            start=True,
                            stop=False,
                            tile_position=(K0 * h, 32 * j),
                        )
                        nc.tensor.matmul(
                            P[32 * j : 32 * j + 32, :],
                            lhs_lo[K0 * h : K0 * h + K0, :],
                            ulo[K0 * h : K0 * h + K0, f0 : f0 + 512],
                            start=False,
                            stop=True,
                            tile_position=(K0 * h, 32 * j),
                        )
                    else:
                        u2 = u_tiles[g]
                        nc.tensor.matmul(
                            P[32 * j : 32 * j + 32, :],
                            lhsT[NB * h : NB * h + NB, :],
                            u2[NB * h : NB * h + NB, f0 : f0 + 512],
                            start=True,
                            stop=True,
                            tile_position=(NB * h, 32 * j),
                        )
                s0 = 512 * (2 * q + h)
                eng = copy_engines[tcount % len(copy_engines)]
                tcount += 1
                if eng is nc.scalar:
                    eng.copy(S[:, s0 : s0 + 512], P[:, :])
                else:
                    eng.tensor_copy(S[:, s0 : s0 + 512], P[:, :])

        # Alternate store queues (sync HWDGE / gpsimd SWDGE); the
        # latency-sensitive final chunk goes on the low-latency sync ring.
        st = nc.sync if (g % 2 == 0 or g == len(CHUNKS) - 1) else nc.gpsimd
        st.dma_start(O_d[g], S[:, :])


def build_nc():
    nc = bacc.Bacc(
        "TRN2",
        target_bir_lowering=False,
        debug=False,
        enable_asserts=False,
        num_devices=C,
        enable_partition_id=False,
    )
    with tile.TileContext(nc) as tc:
        with ExitStack() as ctx:
            _emit(ctx, tc)
    nc.compile()
    return nc


def make_in_maps(z, U, L, mu):
    import ml_dtypes

    z = np.asarray(z, dtype=np.float32)
    U = np.asarray(U, dtype=np.float32)
    L = np.asarray(L, dtype=np.float32).reshape(NB)
    zL = (L[None, :] * z).T  # (64, 32) f32; lhsT[k, b] = L[k] * z[b, k]
    # U scramble: per core/chunk, partition K0*h + r / col f holds quantized
    # row r for half h: U[row, c*CPD + HALF*h + chunk_col0 + f]
    in_maps = []
    if MIXED_FP8:
        zhi = zL[:K0].astype(np.float16)
        zlo = (zL[K0:] / LO_SCALE).astype(np.float16)
        lhs_hi = np.ascontiguousarray(np.concatenate([zhi, zhi], axis=0))
        lhs_lo = np.ascontiguousarray(np.concatenate([zlo, zlo], axis=0))
        Uhi = U[:K0].astype(np.float16)
        Ulo = (U[K0:] * LO_SCALE).astype(ml_dtypes.float8_e4m3fn)
        Vhi = Uhi.reshape(K0, C, 2, HALF)  # r, c, h, f
        Vlo = Ulo.reshape(NB - K0, C, 2, HALF)
        for c in range(C):
            m = {"lhsT_hi": lhs_hi, "lhsT_lo": lhs_lo}
            c0 = 0
            for g, w in enumerate(CHUNKS):
                bh = Vhi[:, c, :, c0 : c0 + w]  # (r, h, w)
                bl = Vlo[:, c, :, c0 : c0 + w]
                m[f"Uhi{g}"] = np.ascontiguousarray(
                    bh.transpose(1, 0, 2).reshape(2 * K0, w)
                )
                m[f"Ulo{g}"] = np.ascontiguousarray(
                    bl.transpose(1, 0, 2).reshape(2 * K0, w)
                )
                c0 += w
            in_maps.append(m)
    else:
        zL16 = zL.astype(np.float16)
        lhsT = np.ascontiguousarray(np.concatenate([zL16, zL16], axis=0))
        U16 = U.astype(np.float16)
        V = U16.reshape(NB, C, 2, HALF)  # k, c, h, f
        for c in range(C):
            m = {"lhsT": lhsT}
            c0 = 0
            for g, w in enumerate(CHUNKS):
                blk = V[:, c, :, c0 : c0 + w]  # (k, h, w)
                m[f"U{g}"] = np.ascontiguousarray(
                    blk.transpose(1, 0, 2).reshape(128, w)
                )
                c0 += w
            in_maps.append(m)
    return in_maps


def get_nc(fast=True):
    if "nc" not in _NC_CACHE:
        _NC_CACHE["nc"] = build_nc()
    return _NC_CACHE["nc"]


def _unscramble(res_c):
    """Per-core chunk outputs -> (B, CPD) f32 projection."""
    proj = np.empty((B, 2, HALF), dtype=np.float32)
    c0 = 0
    for g, w in enumerate(CHUNKS):
        O = res_c[f"O{g}"]  # (128, w//2) fp16
        NQ = w // 2048
        # O[32j+b, 512*(2q+h)+k] = proj[b, h, c0 + 2048q + 512j + k]
        O5 = O.reshape(4, B, NQ, 2, 512)            # j, b, q, h, k
        blk = O5.transpose(1, 3, 2, 0, 4)           # b, h, q, j, k
        proj[:, :, c0 : c0 + w] = blk.reshape(B, 2, w)
        c0 += w
    return proj.reshape(B, CPD)


def kernel(z, U, L, mu):
    nc = get_nc()
    in_maps = make_in_maps(z, U, L, mu)
    res = run_bass_kernel_spmd(nc, in_maps, core_ids=list(range(C)))
    projs = [_unscramble(res.results[c]) for c in range(C)]
    interior = np.stack(projs, axis=1).reshape(B, C, CORE, CORE, CORE)
    mu = np.asarray(mu, dtype=np.float32)
    if np.any(mu):
        interior = interior + mu.reshape(1, C, CORE, CORE, CORE)
    out = np.zeros((B, C, RES, RES, RES), dtype=np.float32)
    out[:, :, POS : POS + CORE, POS : POS + CORE, POS : POS + CORE] = interior
    return out
